# revision 21
# baseline (speedup 1.0000x reference)
# Trainium2 Bass kernel for nn_CosSimRouter_pad.
#
# Strategy (8 NeuronCores, SPMD, no collectives):
#   Device program 1 ("scores"): cos = normalize(vision) @ normalize(text).T
#     sharded over the text dim (1024 text rows per core). Text-stationary
#     matmul: stationary = 128-text-column tiles of the shard (always 128
#     wide, so the compiler's fast-weight-load path engages), moving = the
#     resident normalized vision matrix (576 wide = 512 + 64 psum split).
#     Runs in bf16 or fp8e4+DoubleRow; each core returns its [1024, 576]
#     cos.T shard in bf16. The host takes the noisy top-8 text candidates
#     per vision token and rescores them exactly in fp64, so matmul noise
#     never reaches the (discrete) selection stage.
#   Host: softmax/argsort/cumsum threshold selection, neighbor expansion,
#     unique, then the small [S,576] cos-sim + top-16 + softmax weights are
#     computed with jax on CPU using the exact op sequence of the original
#     module so the selection matches it bit-for-bit. The weights are
#     scattered into a dense row-sparse matrix W [576, 576].
#   Device program 2 ("pool"): out = W @ vision_feature in bf16, sharded
#     over output columns (512 per core), returned bf16, upcast on host.
#
# Both programs start with a short burst of dummy matmuls so the PE HAM
# clock-gate warms up (1.2 -> 2.4 GHz) while the first DMA chunks land.

import os

os.environ.setdefault("MYCRO_LOCAL_CACHE", "1")

import numpy as np
import ml_dtypes

GAMMA = 0.5
TEMP = 0.05
TOP_K = 16
PAD = 1
GRID = 24
EPS = 1e-8

LV = 576          # vision tokens
LT = 8192         # text tokens
D = 4096          # embed dim
NCORES = 8
LT_SH = LT // NCORES          # 1024 text rows per core
KT = D // 128                 # 32 contraction tiles
KT2 = KT // 2                 # 16 contraction pair-tiles (fp8 DoubleRow)
TM = LT_SH // 128             # 8 text tiles of 128 per core
TM_PASSES = ((0, 1, 2, 3), (4, 5, 6, 7))  # 4 tm live = 8 psum banks
M_TILES = (128, 128, 128, 128, 64)   # pool: 576 = 4*128 + 64
KV = 5                        # ceil(576/128) contraction tiles for program 2
NCAND = 8                     # noisy top-k candidates rescored exactly on host
FP8_SCALE = 64.0              # pre-scale so fp8e4 sees ~N(0,1) magnitudes
N_WARM = 6                    # dummy warm-up matmuls per program

# "bf16" or "fp8dr" (fp8e4 + DoubleRow, 2x matmul rate)
SCORES_MODE = "fp8dr"

_cache: dict = {}


def _warmup(nc, wsb_pool, warm_psum, mybir, n_warm=N_WARM):
    """Dummy matmuls at program start: PE busy while first DMAs land, so the
    HAM activity window un-throttles the clock before the real work. Writes
    into a real psum tile; the first real start=True matmul re-zeroes it."""
    wt = wsb_pool.tile([128, 512], mybir.dt.bfloat16)
    nc.vector.memset(wt[:, :], 0)
    for _ in range(n_warm):
        nc.tensor.matmul(
            warm_psum[:, :], lhsT=wt[:, :128], rhs=wt[:, :], start=True, stop=True
        )


def _build_scores_nc(mode: str):
    """Per text shard: full [1024, 576] cos.T matrix in bf16 (text-stationary)."""
    import concourse.mybir as mybir
    import concourse.tile as tile
    from concourse import bacc

    nc = bacc.Bacc(
        "TRN2",
        target_bir_lowering=False,
        debug=False,
        enable_asserts=True,
        num_devices=NCORES,
    )
    bf16 = mybir.dt.bfloat16
    f32 = mybir.dt.float32

    HTM = TM // 2  # 4 text tiles per pass; one DMA trigger covers all 4
    if mode == "bf16":
        mdt = bf16
        # partition-major: each SBUF partition's data is one contiguous DRAM
        # run, and each chunk is a single DMA trigger (the ~700ns per-trigger
        # issue cost on the sync engine was the early-stream bottleneck)
        vnT = nc.dram_tensor("vnT", [128, KT, LV], mdt, kind="ExternalInput").ap()
        tnA = nc.dram_tensor("tnA", [128, KT, HTM * 128], mdt, kind="ExternalInput").ap()
        tnB = nc.dram_tensor("tnB", [128, KT, HTM * 128], mdt, kind="ExternalInput").ap()
        CHUNKS = (1, 1, 2, 4, 8, 8, 8)   # k-tiles per streamed chunk
        NK = KT
    else:  # fp8dr: k-tiles packed in pairs for DoubleRow
        mdt = mybir.dt.float8e4
        vnT = nc.dram_tensor("vnT", [128, KT2, 2, LV], mdt, kind="ExternalInput").ap()
        tnA = nc.dram_tensor(
            "tnA", [128, KT2, 2, HTM * 128], mdt, kind="ExternalInput"
        ).ap()
        tnB = nc.dram_tensor(
            "tnB", [128, KT2, 2, HTM * 128], mdt, kind="ExternalInput"
        ).ap()
        CHUNKS = (1, 1, 1, 1, 2, 2, 4, 4)   # pair-tiles per streamed chunk
        NK = KT2
    assert sum(CHUNKS) == NK

    # sc[tm, t, v] = cos[v, tm*128 + t]
    sc = nc.dram_tensor("sc", [TM, 128, LV], bf16, kind="ExternalOutput").ap()

    with tile.TileContext(nc) as tc:
        with (
            tc.tile_pool(name="warm", bufs=1) as warm_pool,
            tc.tile_pool(name="vn", bufs=1) as vn_pool,
            tc.tile_pool(name="tn", bufs=1) as tn_pool,
            tc.tile_pool(name="ob", bufs=4) as out_pool,
            tc.tile_pool(name="psum", bufs=4, space="PSUM") as psum_pool,
        ):
            if mode == "bf16":
                vn_sb = vn_pool.tile([128, KT, LV], mdt)
                tnA_sb = tn_pool.tile([128, KT, HTM * 128], mdt, name="tnA_sb")
                tnB_sb = tn_pool.tile([128, KT, HTM * 128], mdt, name="tnB_sb")
            else:
                vn_sb = vn_pool.tile([128, KT2, 2, LV], mdt)
                tnA_sb = tn_pool.tile([128, KT2, 2, HTM * 128], mdt, name="tnA_sb")
                tnB_sb = tn_pool.tile([128, KT2, 2, HTM * 128], mdt, name="tnB_sb")
            tn_sbs = (tnA_sb, tnB_sb)
            tn_drams = (tnA, tnB)

            def dma_tn(pi, kc, ch):
                if mode == "bf16":
                    nc.sync.dma_start(
                        tn_sbs[pi][:, kc : kc + ch, :],
                        tn_drams[pi][:, kc : kc + ch, :],
                    )
                else:
                    nc.sync.dma_start(
                        tn_sbs[pi][:, kc : kc + ch, :, :],
                        tn_drams[pi][:, kc : kc + ch, :, :],
                    )

            for pi, tms in enumerate(TM_PASSES):
                # full-bank psum tiles (psB only uses 64 cols): sharing a
                # bank across accumulation groups is unsafe because
                # start=True zeroes at zero-region granularity
                psA = {
                    tm: psum_pool.tile([128, 512], f32, name=f"psA_{tm}", tag="psA")
                    for tm in tms
                }
                psB = {
                    tm: psum_pool.tile([128, 512], f32, name=f"psB_{tm}", tag="psB")
                    for tm in tms
                }
                if pi == 0:
                    _warmup(nc, warm_pool, psA[tms[0]], mybir)
                kc = 0
                for ch in (CHUNKS if pi == 0 else (NK,)):
                    if pi == 0:
                        # vn chunk loads ride the scalar HWDGE queue
                        if mode == "bf16":
                            nc.scalar.dma_start(
                                vn_sb[:, kc : kc + ch, :], vnT[:, kc : kc + ch, :]
                            )
                        else:
                            nc.scalar.dma_start(
                                vn_sb[:, kc : kc + ch, :, :],
                                vnT[:, kc : kc + ch, :, :],
                            )
                        dma_tn(0, kc, ch)
                    for kk in range(ch):
                        k = kc + kk
                        for tmi, tm in enumerate(tms):
                            js = tmi * 128
                            if mode == "bf16":
                                nc.tensor.matmul(
                                    psA[tm][:, 0:512],
                                    lhsT=tn_sbs[pi][:, k, js : js + 128],
                                    rhs=vn_sb[:, k, 0:512],
                                    start=(k == 0),
                                    stop=(k == NK - 1),
                                )
                                nc.tensor.matmul(
                                    psB[tm][:, 0:64],
                                    lhsT=tn_sbs[pi][:, k, js : js + 128],
                                    rhs=vn_sb[:, k, 512:LV],
                                    start=(k == 0),
                                    stop=(k == NK - 1),
                                )
                            else:
                                nc.tensor.matmul(
                                    psA[tm][:, 0:512],
                                    lhsT=tn_sbs[pi][:, k, :, js : js + 128],
                                    rhs=vn_sb[:, k, :, 0:512],
                                    start=(k == 0),
                                    stop=(k == NK - 1),
                                    perf_mode=mybir.MatmulPerfMode.DoubleRow,
                                )
                                nc.tensor.matmul(
                                    psB[tm][:, 0:64],
                                    lhsT=tn_sbs[pi][:, k, :, js : js + 128],
                                    rhs=vn_sb[:, k, :, 512:LV],
                                    start=(k == 0),
                                    stop=(k == NK - 1),
                                    perf_mode=mybir.MatmulPerfMode.DoubleRow,
                                )
                    kc += ch
                if pi == 0:
                    # queue next pass's tn transfers on the sync ring NOW,
                    # ahead of this pass's output DMAs (whose triggers stall
                    # on the last matmul and would block the prefetch)
                    for nch_c in range(0, NK, NK // 2):
                        dma_tn(1, nch_c, NK // 2)
                # downcast finished psums to bf16 and stream out; pass-0
                # copies/DMAs overlap pass-1 matmuls. Casts split across the
                # vector and scalar engines so the final-pass tail is short.
                for i, tm in enumerate(tms):
                    ot = out_pool.tile([128, LV], bf16, name=f"ot_{tm}", tag="ot")
                    if i % 2 == 0:
                        nc.vector.tensor_copy(ot[:, 0:512], psA[tm][:, 0:512])
                        nc.scalar.copy(ot[:, 512:LV], psB[tm][:, 0:64])
                    else:
                        nc.scalar.copy(ot[:, 0:512], psA[tm][:, 0:512])
                        nc.vector.tensor_copy(ot[:, 512:LV], psB[tm][:, 0:64])
                    eng = nc.sync if i % 2 == 0 else nc.scalar
                    eng.dma_start(sc[tm], ot[:, :])

    nc.compile()
    return nc


def _build_pool_nc():
    """out[:, c*512:(c+1)*512] = (W @ vf) for this core's 512-column slice.

    Column sharding: each core gets the full (small) W but only a 512-wide
    slice of vf. All operands bf16; output bf16, upcast on host."""
    import concourse.mybir as mybir
    import concourse.tile as tile
    from concourse import bacc

    nc = bacc.Bacc(
        "TRN2",
        target_bir_lowering=False,
        debug=False,
        enable_asserts=True,
        num_devices=NCORES,
    )
    bf16 = mybir.dt.bfloat16
    f32 = mybir.dt.float32
    # partition-major: one DMA trigger covers all k-tiles of an operand
    wT = nc.dram_tensor("wT", [128, KV, LV], bf16, kind="ExternalInput").ap()
    vf = nc.dram_tensor("vf", [128, KV, 512], bf16, kind="ExternalInput").ap()
    out = nc.dram_tensor("out", [LV, 512], bf16, kind="ExternalOutput").ap()

    with tile.TileContext(nc) as tc:
        with (
            tc.tile_pool(name="warm", bufs=1) as warm_pool,
            tc.tile_pool(name="w", bufs=1) as w_pool,
            tc.tile_pool(name="vfp", bufs=1) as vf_pool,
            tc.tile_pool(name="ob", bufs=5) as out_pool,
            tc.tile_pool(name="psum", bufs=5, space="PSUM") as psum_pool,
        ):
            w_sb = w_pool.tile([128, KV, LV], bf16)
            vf_sb = vf_pool.tile([128, KV, 512], bf16)
            psums = [
                psum_pool.tile([128, 512], f32, name=f"pps{m}", tag="pps")
                for m in range(len(M_TILES))
            ]
            _warmup(nc, warm_pool, psums[0], mybir, n_warm=7)
            # two triggers per operand: k=0 lands early, rest streams behind
            nc.scalar.dma_start(w_sb[:, 0:1, :], wT[:, 0:1, :])
            nc.sync.dma_start(vf_sb[:, 0:1, :], vf[:, 0:1, :])
            nc.scalar.dma_start(w_sb[:, 1:KV, :], wT[:, 1:KV, :])
            nc.sync.dma_start(vf_sb[:, 1:KV, :], vf[:, 1:KV, :])
            for k in range(KV):
                for m, pm in enumerate(M_TILES):
                    nc.tensor.matmul(
                        psums[m][:pm, :],
                        lhsT=w_sb[:, k, m * 128 : m * 128 + pm],
                        rhs=vf_sb[:, k, :],
                        start=(k == 0),
                        stop=(k == KV - 1),
                    )
            for m, pm in enumerate(M_TILES):
                ot = out_pool.tile([128, 512], bf16, name=f"pot{m}", tag="pot")
                if m % 2 == 0:
                    nc.vector.tensor_copy(ot[:pm, :], psums[m][:pm, :])
                else:
                    nc.scalar.copy(ot[:pm, :], psums[m][:pm, :])
                eng = nc.scalar if m % 2 == 0 else nc.sync
                eng.dma_start(out[m * 128 : m * 128 + pm, :], ot[:pm, :])

    nc.compile()
    return nc


def _get_nc(which: str):
    key = which
    if key not in _cache:
        if which == "scores":
            _cache[key] = _build_scores_nc(SCORES_MODE)
        else:
            _cache[key] = _build_pool_nc()
    return _cache[key]


class _Runner:
    """Cached PJRT executor for one Bass program across the 8 cores.

    Mirrors bass2jax.run_bass_via_pjrt's multi-core branch, but builds the
    jitted shard_map once (that function re-traces and re-compiles on every
    call) and lets chosen inputs be replicated instead of concatenated.

    Call with a dict: sharded inputs as global arrays (axis 0 = n_cores *
    per-core axis 0), replicated inputs at their per-core shape. Returns
    {name: global ndarray} with outputs concatenated along axis 0.
    """

    def __init__(self, nc, replicated=()):
        import jax
        from jax.experimental.shard_map import shard_map
        from jax.sharding import Mesh, PartitionSpec

        import concourse.mybir as mybir
        from concourse import bass2jax

        bass2jax.install_neuronx_cc_hook()
        assert not nc.has_collectives and nc.dbg_addr is None
        self.nc = nc
        part_name = nc.partition_id_tensor.name if nc.partition_id_tensor else None
        in_names, out_names, out_avals = [], [], []
        for alloc in nc.m.functions[0].allocations:
            if not isinstance(alloc, mybir.MemoryLocationSet):
                continue
            name = alloc.memorylocations[0].name
            if alloc.kind == "ExternalInput":
                if name != part_name:
                    in_names.append(name)
            elif alloc.kind == "ExternalOutput":
                out_names.append(name)
                out_avals.append(
                    jax.core.ShapedArray(
                        tuple(alloc.tensor_shape), mybir.dt.np(alloc.dtype)
                    )
                )
        self.in_names, self.out_names, self.out_avals = in_names, out_names, out_avals
        self.replicated = set(replicated)
        n_params = len(in_names)
        donate = tuple(range(n_params, n_params + len(out_names)))

        bind_names = in_names + out_names + ([part_name] if part_name else [])

        def _body(*args):
            operands = list(args)
            if part_name is not None:
                operands.append(bass2jax.partition_id_tensor())
            outs = bass2jax._bass_exec_p.bind(
                *operands,
                out_avals=tuple(out_avals),
                in_names=tuple(bind_names),
                out_names=tuple(out_names),
                lowering_input_output_aliases=(),
                sim_require_finite=True,
                sim_require_nnan=True,
                nc=nc,
            )
            return tuple(outs)

        devices = jax.devices()[:NCORES]
        mesh = Mesh(np.asarray(devices), ("core",))
        in_specs = tuple(
            PartitionSpec() if n in self.replicated else PartitionSpec("core")
            for n in in_names
        ) + (PartitionSpec("core"),) * len(out_names)
        out_specs = (PartitionSpec("core"),) * len(out_names)
        self._fn = jax.jit(
            shard_map(
                _body,
                mesh=mesh,
                in_specs=in_specs,
                out_specs=out_specs,
                check_rep=False,
            ),
            donate_argnums=donate,
            keep_unused=True,
        )

    def __call__(self, inputs: dict):
        args = [np.ascontiguousarray(inputs[n]) for n in self.in_names]
        zeros = [
            np.zeros((NCORES * a.shape[0], *a.shape[1:]), a.dtype)
            for a in self.out_avals
        ]
        outs = self._fn(*args, *zeros)
        return {n: np.asarray(o) for n, o in zip(self.out_names, outs)}


_runners: dict = {}


def _get_runner(which: str) -> _Runner:
    key = which
    if key not in _runners:
        repl = {"scores": ("vnT",), "pool": ("wT",)}[which]
        _runners[key] = _Runner(_get_nc(which), replicated=repl)
    return _runners[key]


def _neighbor_unique(sel: np.ndarray) -> np.ndarray:
    offs = np.array(
        [
            [i, j]
            for i in range(-PAD, PAD + 1)
            for j in range(-PAD, PAD + 1)
            if not (i == 0 and j == 0)
        ],
        dtype=np.int64,
    )
    coords = np.stack([sel // GRID, sel % GRID], axis=1)
    padded = np.clip(coords[:, None, :] + offs[None, :, :], 0, GRID - 1)
    return np.unique(padded[..., 0] * GRID + padded[..., 1])


def kernel(vision_feature, text_embed, attention_mask):
    import jax
    import jax.numpy as jnp

    cpu = jax.devices("cpu")[0]

    vision_feature = np.asarray(vision_feature, dtype=np.float32)
    text_embed = np.asarray(text_embed, dtype=np.float32)
    mask_np = np.asarray(attention_mask)

    with jax.default_device(cpu):
        # normalize exactly as the reference does (jnp on CPU)
        vfj = jnp.asarray(vision_feature)
        tej = jnp.asarray(text_embed)
        vn = np.asarray(
            vfj / jnp.maximum(jnp.linalg.norm(vfj, axis=-1, keepdims=True), EPS)
        )
        tn = np.asarray(
            tej / jnp.maximum(jnp.linalg.norm(tej, axis=-1, keepdims=True), EPS)
        )

    # fold the attention mask into the text rows: where(mask, cos, 0) ==
    # cos * mask elementwise, and max over the text dim commutes with the
    # per-vision positive scale, so pre-scaling text rows by mask is exact.
    tns = tn * mask_np.astype(np.float32)[:, None]

    # ---- device program 1: sharded cos-sim, full bf16 cos.T shards ----
    if SCORES_MODE == "bf16":
        mnp = ml_dtypes.bfloat16
        vnT = np.ascontiguousarray(
            vn.T.reshape(KT, 128, LV).transpose(1, 0, 2)
        ).astype(mnp)
        # tn5[c, p, k, tm, j] = tns[c*1024 + tm*128 + j, k*128 + p]
        tn5 = tns.reshape(NCORES, TM, 128, KT, 128).transpose(0, 4, 3, 1, 2)
        tnA_g = np.ascontiguousarray(tn5[:, :, :, 0:4, :]).reshape(
            NCORES * 128, KT, 512
        ).astype(mnp)
        tnB_g = np.ascontiguousarray(tn5[:, :, :, 4:8, :]).reshape(
            NCORES * 128, KT, 512
        ).astype(mnp)
    else:
        mnp = ml_dtypes.float8_e4m3
        vnT = np.ascontiguousarray(
            (vn.T * FP8_SCALE).reshape(KT2, 2, 128, LV).transpose(2, 0, 1, 3)
        ).astype(mnp)
        # tn6[c, p, t, i, tm, j] = tns[c*1024+tm*128+j, (t*2+i)*128 + p] * S
        tn6 = (tns * FP8_SCALE).reshape(NCORES, TM, 128, KT2, 2, 128).transpose(
            0, 5, 3, 4, 1, 2
        )
        tnA_g = np.ascontiguousarray(tn6[:, :, :, :, 0:4, :]).reshape(
            NCORES * 128, KT2, 2, 512
        ).astype(mnp)
        tnB_g = np.ascontiguousarray(tn6[:, :, :, :, 4:8, :]).reshape(
            NCORES * 128, KT2, 2, 512
        ).astype(mnp)

    out1 = _get_runner("scores")({"vnT": vnT, "tnA": tnA_g, "tnB": tnB_g})
    # sc[c, tm, t, v] -> cos[v, c*1024 + tm*128 + t]
    sc = out1["sc"].reshape(NCORES, TM, 128, LV)
    cos_noisy = sc.transpose(3, 0, 1, 2).reshape(LV, LT).astype(np.float32)

    # noisy top-NCAND text candidates per vision token, rescored exactly in
    # fp64: the true argmax is inside the noisy top-8 by a wide margin, so
    # the selection sees bit-exact scores regardless of matmul precision.
    cand = np.argpartition(-cos_noisy, NCAND - 1, axis=1)[:, :NCAND]  # [LV, NCAND]
    exact = np.einsum(
        "mkd,md->mk", tns[cand].astype(np.float64), vn.astype(np.float64)
    ).astype(np.float32)
    scores = exact.max(axis=1)  # [LV]

    # ---- host selection (mirrors reference ops; margins >> fp32 noise) ----
    with jax.default_device(cpu):
        sj = jnp.asarray(scores)
        probs = jax.nn.softmax(sj / TEMP)
        order = jnp.argsort(-probs)
        cum = jnp.cumsum(probs[order])
        thr = int(jnp.sum(cum <= GAMMA))
        sel = np.asarray(order[:thr])

    if thr == 0:
        return np.zeros((0, D), dtype=np.float32)
    uniq = _neighbor_unique(sel)
    S = len(uniq)

    # ---- host: small [S,576] cos-sim + top-k + softmax, bit-exact ----
    with jax.default_device(cpu):
        sel_feat = jnp.asarray(vision_feature[uniq])
        sn = sel_feat / jnp.maximum(
            jnp.linalg.norm(sel_feat, axis=-1, keepdims=True), EPS
        )
        scos = sn @ jnp.asarray(vn).T
        top_vals, top_idx = jax.lax.top_k(scos, TOP_K)
        w = np.asarray(jax.nn.softmax(top_vals, axis=-1))
        top_idx = np.asarray(top_idx)

    W = np.zeros((LV, LV), dtype=np.float32)  # rows: uniq order; cols: vision j
    W[np.arange(S)[:, None], top_idx] = w

    # ---- device program 2: out = W @ vision_feature, column-sharded ----
    WT = np.zeros((KV * 128, LV), dtype=np.float32)
    WT[:LV] = W.T
    # wT[p, k, j] = W[j, k*128+p], replicated
    wT_r = np.ascontiguousarray(
        WT.reshape(KV, 128, LV).transpose(1, 0, 2)
    ).astype(ml_dtypes.bfloat16)
    vf_p = np.zeros((KV * 128, D), dtype=np.float32)
    vf_p[:LV] = vision_feature
    # global vf[c*128+p, k, j] = vf_p[k*128+p, c*512+j]
    vf_g = np.ascontiguousarray(
        vf_p.reshape(KV, 128, NCORES, 512).transpose(2, 1, 0, 3)
    ).reshape(NCORES * 128, KV, 512).astype(ml_dtypes.bfloat16)

    out2 = _get_runner("pool")({"wT": wT_r, "vf": vf_g})
    # out is [NCORES*576, 512]: per-core column slices of [576, 4096]
    out_full = (
        out2["out"].reshape(NCORES, LV, 512).transpose(1, 0, 2)
        .reshape(LV, D).astype(np.float32)
    )
    return np.ascontiguousarray(out_full[:S])


# revision 25
# speedup vs baseline: 1.4787x; 1.4787x over previous
# Trainium2 Bass kernel for nn_CosSimRouter_pad.
#
# Strategy (8 NeuronCores, SPMD, no collectives):
#   Device program 1 ("scores"): cos = normalize(vision) @ normalize(text).T
#     sharded over the text dim (1024 text rows per core). Text-stationary
#     matmul: stationary = 128-text-column tiles of the shard (always 128
#     wide, so the compiler's fast-weight-load path engages), moving = the
#     resident normalized vision matrix (576 wide = 512 + 64 psum split).
#     Runs in bf16 or fp8e4+DoubleRow; each core returns its [1024, 576]
#     cos.T shard in bf16. The host takes the noisy top-8 text candidates
#     per vision token and rescores them exactly in fp64, so matmul noise
#     never reaches the (discrete) selection stage.
#   Host: softmax/argsort/cumsum threshold selection, neighbor expansion,
#     unique, then the small [S,576] cos-sim + top-16 + softmax weights are
#     computed with jax on CPU using the exact op sequence of the original
#     module so the selection matches it bit-for-bit. The weights are
#     scattered into a dense row-sparse matrix W [576, 576].
#   Device program 2 ("pool"): out = W @ vision_feature in bf16, sharded
#     over output columns (512 per core), returned bf16, upcast on host.
#
# Both programs start with a short burst of dummy matmuls so the PE HAM
# clock-gate warms up (1.2 -> 2.4 GHz) while the first DMA chunks land.

import os

os.environ.setdefault("MYCRO_LOCAL_CACHE", "1")

import numpy as np
import ml_dtypes

GAMMA = 0.5
TEMP = 0.05
TOP_K = 16
PAD = 1
GRID = 24
EPS = 1e-8

LV = 576          # vision tokens
LT = 8192         # text tokens
D = 4096          # embed dim
NCORES = 8
LT_SH = LT // NCORES          # 1024 text rows per core
KT = D // 128                 # 32 contraction tiles
KT2 = KT // 2                 # 16 contraction pair-tiles (fp8 DoubleRow)
TM = LT_SH // 128             # 8 text tiles of 128 per core
TM_PASSES = ((0, 1, 2, 3), (4, 5, 6, 7))  # 4 tm live = 8 psum banks
M_TILES = (128, 128, 128, 128, 64)   # pool: 576 = 4*128 + 64
KV = 5                        # ceil(576/128) contraction tiles for program 2
NCAND = 8                     # noisy top-k candidates rescored exactly on host
FP8_SCALE = 64.0              # pre-scale so fp8e4 sees ~N(0,1) magnitudes
N_WARM = 4                    # dummy warm-up matmuls per program

# If False, the small final pooling (W @ vision_feature, 2.7 GFLOP = 6.5% of
# the module's FLOPs) is done on host in exact fp32 as part of the
# selection/gather stage; the device runs the dominant cos-sim matmul.
POOL_ON_DEVICE = False

# "bf16" or "fp8dr" (fp8e4 + DoubleRow, 2x matmul rate)
SCORES_MODE = "fp8dr"

_cache: dict = {}


def _warmup(nc, wsb_pool, warm_psum, mybir, n_warm=N_WARM):
    """Dummy matmuls at program start: PE busy while first DMAs land, so the
    HAM activity window un-throttles the clock before the real work. Writes
    into a real psum tile; the first real start=True matmul re-zeroes it."""
    wt = wsb_pool.tile([128, 512], mybir.dt.bfloat16)
    nc.vector.memset(wt[:, :], 0)
    for _ in range(n_warm):
        nc.tensor.matmul(
            warm_psum[:, :], lhsT=wt[:, :128], rhs=wt[:, :], start=True, stop=True
        )


def _build_scores_nc(mode: str):
    """Per text shard: full [1024, 576] cos.T matrix in bf16 (text-stationary)."""
    import concourse.mybir as mybir
    import concourse.tile as tile
    from concourse import bacc

    nc = bacc.Bacc(
        "TRN2",
        target_bir_lowering=False,
        debug=False,
        enable_asserts=True,
        num_devices=NCORES,
    )
    bf16 = mybir.dt.bfloat16
    f32 = mybir.dt.float32

    HTM = TM // 2  # 4 text tiles per pass; one DMA trigger covers all 4
    if mode == "bf16":
        mdt = bf16
        # partition-major: each SBUF partition's data is one contiguous DRAM
        # run, and each chunk is a single DMA trigger (the ~700ns per-trigger
        # issue cost on the sync engine was the early-stream bottleneck)
        vnT = nc.dram_tensor("vnT", [128, KT, LV], mdt, kind="ExternalInput").ap()
        tnA = nc.dram_tensor("tnA", [128, KT, HTM * 128], mdt, kind="ExternalInput").ap()
        tnB = nc.dram_tensor("tnB", [128, KT, HTM * 128], mdt, kind="ExternalInput").ap()
        CHUNKS = (1, 1, 2, 4, 8, 8, 8)   # k-tiles per streamed chunk
        NK = KT
    else:  # fp8dr: k-tiles packed in pairs for DoubleRow
        mdt = mybir.dt.float8e4
        vnT = nc.dram_tensor("vnT", [128, KT2, 2, LV], mdt, kind="ExternalInput").ap()
        tnA = nc.dram_tensor(
            "tnA", [128, KT2, 2, HTM * 128], mdt, kind="ExternalInput"
        ).ap()
        tnB = nc.dram_tensor(
            "tnB", [128, KT2, 2, HTM * 128], mdt, kind="ExternalInput"
        ).ap()
        CHUNKS = (1, 1, 1, 1, 2, 2, 4, 4)   # pair-tiles per streamed chunk
        NK = KT2
    assert sum(CHUNKS) == NK

    # sc[tm, t, v] = cos[v, tm*128 + t]
    sc = nc.dram_tensor("sc", [TM, 128, LV], bf16, kind="ExternalOutput").ap()

    with tile.TileContext(nc) as tc:
        with (
            tc.tile_pool(name="warm", bufs=1) as warm_pool,
            tc.tile_pool(name="vn", bufs=1) as vn_pool,
            tc.tile_pool(name="tn", bufs=1) as tn_pool,
            tc.tile_pool(name="ob", bufs=4) as out_pool,
            tc.tile_pool(name="psum", bufs=4, space="PSUM") as psum_pool,
        ):
            if mode == "bf16":
                vn_sb = vn_pool.tile([128, KT, LV], mdt)
                tnA_sb = tn_pool.tile([128, KT, HTM * 128], mdt, name="tnA_sb")
                tnB_sb = tn_pool.tile([128, KT, HTM * 128], mdt, name="tnB_sb")
            else:
                vn_sb = vn_pool.tile([128, KT2, 2, LV], mdt)
                tnA_sb = tn_pool.tile([128, KT2, 2, HTM * 128], mdt, name="tnA_sb")
                tnB_sb = tn_pool.tile([128, KT2, 2, HTM * 128], mdt, name="tnB_sb")
            tn_sbs = (tnA_sb, tnB_sb)
            tn_drams = (tnA, tnB)

            def dma_tn(pi, kc, ch):
                if mode == "bf16":
                    nc.sync.dma_start(
                        tn_sbs[pi][:, kc : kc + ch, :],
                        tn_drams[pi][:, kc : kc + ch, :],
                    )
                else:
                    nc.sync.dma_start(
                        tn_sbs[pi][:, kc : kc + ch, :, :],
                        tn_drams[pi][:, kc : kc + ch, :, :],
                    )

            for pi, tms in enumerate(TM_PASSES):
                # full-bank psum tiles (psB only uses 64 cols): sharing a
                # bank across accumulation groups is unsafe because
                # start=True zeroes at zero-region granularity
                psA = {
                    tm: psum_pool.tile([128, 512], f32, name=f"psA_{tm}", tag="psA")
                    for tm in tms
                }
                psB = {
                    tm: psum_pool.tile([128, 512], f32, name=f"psB_{tm}", tag="psB")
                    for tm in tms
                }
                if pi == 0:
                    _warmup(nc, warm_pool, psA[tms[0]], mybir)
                kc = 0
                for ch in (CHUNKS if pi == 0 else (NK,)):
                    if pi == 0:
                        # vn chunk loads ride the scalar HWDGE queue
                        if mode == "bf16":
                            nc.scalar.dma_start(
                                vn_sb[:, kc : kc + ch, :], vnT[:, kc : kc + ch, :]
                            )
                        else:
                            nc.scalar.dma_start(
                                vn_sb[:, kc : kc + ch, :, :],
                                vnT[:, kc : kc + ch, :, :],
                            )
                        dma_tn(0, kc, ch)
                    for kk in range(ch):
                        k = kc + kk
                        for tmi, tm in enumerate(tms):
                            js = tmi * 128
                            if mode == "bf16":
                                nc.tensor.matmul(
                                    psA[tm][:, 0:512],
                                    lhsT=tn_sbs[pi][:, k, js : js + 128],
                                    rhs=vn_sb[:, k, 0:512],
                                    start=(k == 0),
                                    stop=(k == NK - 1),
                                )
                                nc.tensor.matmul(
                                    psB[tm][:, 0:64],
                                    lhsT=tn_sbs[pi][:, k, js : js + 128],
                                    rhs=vn_sb[:, k, 512:LV],
                                    start=(k == 0),
                                    stop=(k == NK - 1),
                                )
                            else:
                                nc.tensor.matmul(
                                    psA[tm][:, 0:512],
                                    lhsT=tn_sbs[pi][:, k, :, js : js + 128],
                                    rhs=vn_sb[:, k, :, 0:512],
                                    start=(k == 0),
                                    stop=(k == NK - 1),
                                    perf_mode=mybir.MatmulPerfMode.DoubleRow,
                                )
                                nc.tensor.matmul(
                                    psB[tm][:, 0:64],
                                    lhsT=tn_sbs[pi][:, k, :, js : js + 128],
                                    rhs=vn_sb[:, k, :, 512:LV],
                                    start=(k == 0),
                                    stop=(k == NK - 1),
                                    perf_mode=mybir.MatmulPerfMode.DoubleRow,
                                )
                    kc += ch
                if pi == 0:
                    # queue next pass's tn transfers on the sync ring NOW,
                    # ahead of this pass's output DMAs (whose triggers stall
                    # on the last matmul and would block the prefetch)
                    for nch_c in range(0, NK, NK // 2):
                        dma_tn(1, nch_c, NK // 2)
                # downcast finished psums to bf16 and stream out; pass-0
                # copies/DMAs overlap pass-1 matmuls. Casts split across the
                # vector and scalar engines so the final-pass tail is short.
                for i, tm in enumerate(tms):
                    ot = out_pool.tile([128, LV], bf16, name=f"ot_{tm}", tag="ot")
                    if i % 2 == 0:
                        nc.vector.tensor_copy(ot[:, 0:512], psA[tm][:, 0:512])
                        nc.scalar.copy(ot[:, 512:LV], psB[tm][:, 0:64])
                    else:
                        nc.scalar.copy(ot[:, 0:512], psA[tm][:, 0:512])
                        nc.vector.tensor_copy(ot[:, 512:LV], psB[tm][:, 0:64])
                    eng = nc.sync if i % 2 == 0 else nc.scalar
                    eng.dma_start(sc[tm], ot[:, :])

    nc.compile()
    return nc


def _build_pool_nc():
    """out[:, c*512:(c+1)*512] = (W @ vf) for this core's 512-column slice.

    Column sharding: each core gets the full (small) W but only a 512-wide
    slice of vf. All operands bf16; output bf16, upcast on host."""
    import concourse.mybir as mybir
    import concourse.tile as tile
    from concourse import bacc

    nc = bacc.Bacc(
        "TRN2",
        target_bir_lowering=False,
        debug=False,
        enable_asserts=True,
        num_devices=NCORES,
    )
    bf16 = mybir.dt.bfloat16
    f32 = mybir.dt.float32
    wT = nc.dram_tensor("wT", [KV, 128, LV], bf16, kind="ExternalInput").ap()
    vf = nc.dram_tensor("vf", [KV, 128, 512], bf16, kind="ExternalInput").ap()
    out = nc.dram_tensor("out", [LV, 512], bf16, kind="ExternalOutput").ap()

    with tile.TileContext(nc) as tc:
        with (
            tc.tile_pool(name="warm", bufs=1) as warm_pool,
            tc.tile_pool(name="w", bufs=1) as w_pool,
            tc.tile_pool(name="vfp", bufs=1) as vf_pool,
            tc.tile_pool(name="ob", bufs=5) as out_pool,
            tc.tile_pool(name="psum", bufs=5, space="PSUM") as psum_pool,
        ):
            w_sb = w_pool.tile([128, KV, LV], bf16)
            vf_sb = vf_pool.tile([128, KV, 512], bf16)
            psums = [
                psum_pool.tile([128, 512], f32, name=f"pps{m}", tag="pps")
                for m in range(len(M_TILES))
            ]
            _warmup(nc, warm_pool, psums[0], mybir, n_warm=7)
            # k-outer streaming: first k-tile lands, all 5 m-tiles consume it
            for k in range(KV):
                nc.scalar.dma_start(w_sb[:, k, :], wT[k])
                nc.sync.dma_start(vf_sb[:, k, :], vf[k])
                for m, pm in enumerate(M_TILES):
                    nc.tensor.matmul(
                        psums[m][:pm, :],
                        lhsT=w_sb[:, k, m * 128 : m * 128 + pm],
                        rhs=vf_sb[:, k, :],
                        start=(k == 0),
                        stop=(k == KV - 1),
                    )
            for m, pm in enumerate(M_TILES):
                ot = out_pool.tile([128, 512], bf16, name=f"pot{m}", tag="pot")
                if m % 2 == 0:
                    nc.vector.tensor_copy(ot[:pm, :], psums[m][:pm, :])
                else:
                    nc.scalar.copy(ot[:pm, :], psums[m][:pm, :])
                eng = nc.scalar if m % 2 == 0 else nc.sync
                eng.dma_start(out[m * 128 : m * 128 + pm, :], ot[:pm, :])

    nc.compile()
    return nc


def _get_nc(which: str):
    key = which
    if key not in _cache:
        if which == "scores":
            _cache[key] = _build_scores_nc(SCORES_MODE)
        else:
            _cache[key] = _build_pool_nc()
    return _cache[key]


class _Runner:
    """Cached PJRT executor for one Bass program across the 8 cores.

    Mirrors bass2jax.run_bass_via_pjrt's multi-core branch, but builds the
    jitted shard_map once (that function re-traces and re-compiles on every
    call) and lets chosen inputs be replicated instead of concatenated.

    Call with a dict: sharded inputs as global arrays (axis 0 = n_cores *
    per-core axis 0), replicated inputs at their per-core shape. Returns
    {name: global ndarray} with outputs concatenated along axis 0.
    """

    def __init__(self, nc, replicated=()):
        import jax
        from jax.experimental.shard_map import shard_map
        from jax.sharding import Mesh, PartitionSpec

        import concourse.mybir as mybir
        from concourse import bass2jax

        bass2jax.install_neuronx_cc_hook()
        assert not nc.has_collectives and nc.dbg_addr is None
        self.nc = nc
        part_name = nc.partition_id_tensor.name if nc.partition_id_tensor else None
        in_names, out_names, out_avals = [], [], []
        for alloc in nc.m.functions[0].allocations:
            if not isinstance(alloc, mybir.MemoryLocationSet):
                continue
            name = alloc.memorylocations[0].name
            if alloc.kind == "ExternalInput":
                if name != part_name:
                    in_names.append(name)
            elif alloc.kind == "ExternalOutput":
                out_names.append(name)
                out_avals.append(
                    jax.core.ShapedArray(
                        tuple(alloc.tensor_shape), mybir.dt.np(alloc.dtype)
                    )
                )
        self.in_names, self.out_names, self.out_avals = in_names, out_names, out_avals
        self.replicated = set(replicated)
        n_params = len(in_names)
        donate = tuple(range(n_params, n_params + len(out_names)))

        bind_names = in_names + out_names + ([part_name] if part_name else [])

        def _body(*args):
            operands = list(args)
            if part_name is not None:
                operands.append(bass2jax.partition_id_tensor())
            outs = bass2jax._bass_exec_p.bind(
                *operands,
                out_avals=tuple(out_avals),
                in_names=tuple(bind_names),
                out_names=tuple(out_names),
                lowering_input_output_aliases=(),
                sim_require_finite=True,
                sim_require_nnan=True,
                nc=nc,
            )
            return tuple(outs)

        devices = jax.devices()[:NCORES]
        mesh = Mesh(np.asarray(devices), ("core",))
        in_specs = tuple(
            PartitionSpec() if n in self.replicated else PartitionSpec("core")
            for n in in_names
        ) + (PartitionSpec("core"),) * len(out_names)
        out_specs = (PartitionSpec("core"),) * len(out_names)
        self._fn = jax.jit(
            shard_map(
                _body,
                mesh=mesh,
                in_specs=in_specs,
                out_specs=out_specs,
                check_rep=False,
            ),
            donate_argnums=donate,
            keep_unused=True,
        )

    def __call__(self, inputs: dict):
        args = [np.ascontiguousarray(inputs[n]) for n in self.in_names]
        zeros = [
            np.zeros((NCORES * a.shape[0], *a.shape[1:]), a.dtype)
            for a in self.out_avals
        ]
        outs = self._fn(*args, *zeros)
        return {n: np.asarray(o) for n, o in zip(self.out_names, outs)}


_runners: dict = {}


def _get_runner(which: str) -> _Runner:
    key = which
    if key not in _runners:
        repl = {"scores": ("vnT",), "pool": ("wT",)}[which]
        _runners[key] = _Runner(_get_nc(which), replicated=repl)
    return _runners[key]


def _neighbor_unique(sel: np.ndarray) -> np.ndarray:
    offs = np.array(
        [
            [i, j]
            for i in range(-PAD, PAD + 1)
            for j in range(-PAD, PAD + 1)
            if not (i == 0 and j == 0)
        ],
        dtype=np.int64,
    )
    coords = np.stack([sel // GRID, sel % GRID], axis=1)
    padded = np.clip(coords[:, None, :] + offs[None, :, :], 0, GRID - 1)
    return np.unique(padded[..., 0] * GRID + padded[..., 1])


def kernel(vision_feature, text_embed, attention_mask):
    import jax
    import jax.numpy as jnp

    cpu = jax.devices("cpu")[0]

    vision_feature = np.asarray(vision_feature, dtype=np.float32)
    text_embed = np.asarray(text_embed, dtype=np.float32)
    mask_np = np.asarray(attention_mask)

    with jax.default_device(cpu):
        # normalize exactly as the reference does (jnp on CPU)
        vfj = jnp.asarray(vision_feature)
        tej = jnp.asarray(text_embed)
        vn = np.asarray(
            vfj / jnp.maximum(jnp.linalg.norm(vfj, axis=-1, keepdims=True), EPS)
        )
        tn = np.asarray(
            tej / jnp.maximum(jnp.linalg.norm(tej, axis=-1, keepdims=True), EPS)
        )

    # fold the attention mask into the text rows: where(mask, cos, 0) ==
    # cos * mask elementwise, and max over the text dim commutes with the
    # per-vision positive scale, so pre-scaling text rows by mask is exact.
    tns = tn * mask_np.astype(np.float32)[:, None]

    # ---- device program 1: sharded cos-sim, full bf16 cos.T shards ----
    if SCORES_MODE == "bf16":
        mnp = ml_dtypes.bfloat16
        vnT = np.ascontiguousarray(
            vn.T.reshape(KT, 128, LV).transpose(1, 0, 2)
        ).astype(mnp)
        # tn5[c, p, k, tm, j] = tns[c*1024 + tm*128 + j, k*128 + p]
        tn5 = tns.reshape(NCORES, TM, 128, KT, 128).transpose(0, 4, 3, 1, 2)
        tnA_g = np.ascontiguousarray(tn5[:, :, :, 0:4, :]).reshape(
            NCORES * 128, KT, 512
        ).astype(mnp)
        tnB_g = np.ascontiguousarray(tn5[:, :, :, 4:8, :]).reshape(
            NCORES * 128, KT, 512
        ).astype(mnp)
    else:
        mnp = ml_dtypes.float8_e4m3
        vnT = np.ascontiguousarray(
            (vn.T * FP8_SCALE).reshape(KT2, 2, 128, LV).transpose(2, 0, 1, 3)
        ).astype(mnp)
        # tn6[c, p, t, i, tm, j] = tns[c*1024+tm*128+j, (t*2+i)*128 + p] * S
        tn6 = (tns * FP8_SCALE).reshape(NCORES, TM, 128, KT2, 2, 128).transpose(
            0, 5, 3, 4, 1, 2
        )
        tnA_g = np.ascontiguousarray(tn6[:, :, :, :, 0:4, :]).reshape(
            NCORES * 128, KT2, 2, 512
        ).astype(mnp)
        tnB_g = np.ascontiguousarray(tn6[:, :, :, :, 4:8, :]).reshape(
            NCORES * 128, KT2, 2, 512
        ).astype(mnp)

    out1 = _get_runner("scores")({"vnT": vnT, "tnA": tnA_g, "tnB": tnB_g})
    # sc[c, tm, t, v] -> cos[v, c*1024 + tm*128 + t]
    sc = out1["sc"].reshape(NCORES, TM, 128, LV)
    cos_noisy = sc.transpose(3, 0, 1, 2).reshape(LV, LT).astype(np.float32)

    # noisy top-NCAND text candidates per vision token, rescored exactly in
    # fp64: the true argmax is inside the noisy top-8 by a wide margin, so
    # the selection sees bit-exact scores regardless of matmul precision.
    cand = np.argpartition(-cos_noisy, NCAND - 1, axis=1)[:, :NCAND]  # [LV, NCAND]
    exact = np.einsum(
        "mkd,md->mk", tns[cand].astype(np.float64), vn.astype(np.float64)
    ).astype(np.float32)
    scores = exact.max(axis=1)  # [LV]

    # ---- host selection (mirrors reference ops; margins >> fp32 noise) ----
    with jax.default_device(cpu):
        sj = jnp.asarray(scores)
        probs = jax.nn.softmax(sj / TEMP)
        order = jnp.argsort(-probs)
        cum = jnp.cumsum(probs[order])
        thr = int(jnp.sum(cum <= GAMMA))
        sel = np.asarray(order[:thr])

    if thr == 0:
        return np.zeros((0, D), dtype=np.float32)
    uniq = _neighbor_unique(sel)
    S = len(uniq)

    # ---- host: small [S,576] cos-sim + top-k + softmax, bit-exact ----
    with jax.default_device(cpu):
        sel_feat = jnp.asarray(vision_feature[uniq])
        sn = sel_feat / jnp.maximum(
            jnp.linalg.norm(sel_feat, axis=-1, keepdims=True), EPS
        )
        scos = sn @ jnp.asarray(vn).T
        top_vals, top_idx = jax.lax.top_k(scos, TOP_K)
        w = np.asarray(jax.nn.softmax(top_vals, axis=-1))
        top_idx = np.asarray(top_idx)

    if not POOL_ON_DEVICE:
        # weighted pooling on host, exact fp32, same op order as the module
        return np.ascontiguousarray(
            (vision_feature[top_idx] * w[..., None]).sum(axis=1)
        )

    W = np.zeros((LV, LV), dtype=np.float32)  # rows: uniq order; cols: vision j
    W[np.arange(S)[:, None], top_idx] = w

    # ---- device program 2: out = W @ vision_feature, column-sharded ----
    WT = np.zeros((KV * 128, LV), dtype=np.float32)
    WT[:LV] = W.T
    wT_r = WT.reshape(KV, 128, LV).astype(ml_dtypes.bfloat16)  # replicated
    vf_p = np.zeros((KV * 128, D), dtype=np.float32)
    vf_p[:LV] = vision_feature
    # global vf[c*KV+k, p, j] = vf_p[k*128+p, c*512+j]
    vf_g = np.ascontiguousarray(
        vf_p.reshape(KV, 128, NCORES, 512).transpose(2, 0, 1, 3)
    ).reshape(NCORES * KV, 128, 512).astype(ml_dtypes.bfloat16)

    out2 = _get_runner("pool")({"wT": wT_r, "vf": vf_g})
    # out is [NCORES*576, 512]: per-core column slices of [576, 4096]
    out_full = (
        out2["out"].reshape(NCORES, LV, 512).transpose(1, 0, 2)
        .reshape(LV, D).astype(np.float32)
    )
    return np.ascontiguousarray(out_full[:S])


# revision 31
# speedup vs baseline: 1.5256x; 1.0317x over previous
# Trainium2 Bass kernel for nn_CosSimRouter_pad.
#
# Strategy (8 NeuronCores, SPMD, no collectives):
#   Device program 1 ("scores"): cos = normalize(vision) @ normalize(text).T
#     sharded over the text dim (1024 text rows per core). Text-stationary
#     matmul: stationary = 128-text-column tiles of the shard (always 128
#     wide, so the compiler's fast-weight-load path engages), moving = the
#     resident normalized vision matrix (576 wide = 512 + 64 psum split).
#     Runs in bf16 or fp8e4+DoubleRow; each core returns its [1024, 576]
#     cos.T shard in bf16. The host takes the noisy top-8 text candidates
#     per vision token and rescores them exactly in fp64, so matmul noise
#     never reaches the (discrete) selection stage.
#   Host: softmax/argsort/cumsum threshold selection, neighbor expansion,
#     unique, then the small [S,576] cos-sim + top-16 + softmax weights are
#     computed with jax on CPU using the exact op sequence of the original
#     module so the selection matches it bit-for-bit. The weights are
#     scattered into a dense row-sparse matrix W [576, 576].
#   Device program 2 ("pool"): out = W @ vision_feature in bf16, sharded
#     over output columns (512 per core), returned bf16, upcast on host.
#
# Both programs start with a short burst of dummy matmuls so the PE HAM
# clock-gate warms up (1.2 -> 2.4 GHz) while the first DMA chunks land.

import os

os.environ.setdefault("MYCRO_LOCAL_CACHE", "1")

import numpy as np
import ml_dtypes

GAMMA = 0.5
TEMP = 0.05
TOP_K = 16
PAD = 1
GRID = 24
EPS = 1e-8

LV = 576          # vision tokens
LT = 8192         # text tokens
D = 4096          # embed dim
NCORES = 8
LT_SH = LT // NCORES          # 1024 text rows per core
KT = D // 128                 # 32 contraction tiles
KT2 = KT // 2                 # 16 contraction pair-tiles (fp8 DoubleRow)
TM = LT_SH // 128             # 8 text tiles of 128 per core
# pass sizes descending: the last pass's cast+DMA tail covers one tile only
TM_PASSES = ((0, 1, 2, 3), (4, 5, 6), (7,))
M_TILES = (128, 128, 128, 128, 64)   # pool: 576 = 4*128 + 64
KV = 5                        # ceil(576/128) contraction tiles for program 2
NCAND = 8                     # noisy top-k candidates rescored exactly on host
FP8_SCALE = 64.0              # pre-scale so fp8e4 sees ~N(0,1) magnitudes
N_WARM = 4                    # dummy warm-up matmuls per program

# If False, the small final pooling (W @ vision_feature, 2.7 GFLOP = 6.5% of
# the module's FLOPs) is done on host in exact fp32 as part of the
# selection/gather stage; the device runs the dominant cos-sim matmul.
POOL_ON_DEVICE = False

# "bf16" or "fp8dr" (fp8e4 + DoubleRow, 2x matmul rate)
SCORES_MODE = "fp8dr"

_cache: dict = {}


def _warmup(nc, wsb_pool, warm_psum, mybir, n_warm=N_WARM):
    """Dummy matmuls at program start: PE busy while first DMAs land, so the
    HAM activity window un-throttles the clock before the real work. Writes
    into a real psum tile; the first real start=True matmul re-zeroes it."""
    wt = wsb_pool.tile([128, 512], mybir.dt.bfloat16)
    nc.vector.memset(wt[:, :], 0)
    for _ in range(n_warm):
        nc.tensor.matmul(
            warm_psum[:, :], lhsT=wt[:, :128], rhs=wt[:, :], start=True, stop=True
        )


def _build_scores_nc(mode: str):
    """Per text shard: full [1024, 576] cos.T matrix in bf16 (text-stationary)."""
    import concourse.mybir as mybir
    import concourse.tile as tile
    from concourse import bacc

    nc = bacc.Bacc(
        "TRN2",
        target_bir_lowering=False,
        debug=False,
        enable_asserts=True,
        num_devices=NCORES,
    )
    bf16 = mybir.dt.bfloat16
    f32 = mybir.dt.float32

    WIDTHS = tuple(len(tms) * 128 for tms in TM_PASSES)
    if mode == "bf16":
        mdt = bf16
        # partition-major: each SBUF partition's data is one contiguous DRAM
        # run, and each chunk is a single DMA trigger (the ~700ns per-trigger
        # issue cost on the sync engine was the early-stream bottleneck)
        vnT = nc.dram_tensor("vnT", [128, KT, LV], mdt, kind="ExternalInput").ap()
        tn_drams = [
            nc.dram_tensor(f"tn{p}", [128, KT, wd], mdt, kind="ExternalInput").ap()
            for p, wd in enumerate(WIDTHS)
        ]
        CHUNKS = (1, 1, 2, 4, 8, 8, 8)   # k-tiles per streamed chunk
        NK = KT
    else:  # fp8dr: k-tiles packed in pairs for DoubleRow
        mdt = mybir.dt.float8e4
        vnT = nc.dram_tensor("vnT", [128, KT2, 2, LV], mdt, kind="ExternalInput").ap()
        tn_drams = [
            nc.dram_tensor(
                f"tn{p}", [128, KT2, 2, wd], mdt, kind="ExternalInput"
            ).ap()
            for p, wd in enumerate(WIDTHS)
        ]
        CHUNKS = (1, 1, 1, 1, 2, 2, 2, 2, 4)   # pair-tiles per streamed chunk
        NK = KT2
    assert sum(CHUNKS) == NK

    # sc[tm, t, v] = cos[v, tm*128 + t]
    sc = nc.dram_tensor("sc", [TM, 128, LV], bf16, kind="ExternalOutput").ap()

    with tile.TileContext(nc) as tc:
        with (
            tc.tile_pool(name="warm", bufs=1) as warm_pool,
            tc.tile_pool(name="vn", bufs=1) as vn_pool,
            tc.tile_pool(name="tn", bufs=1) as tn_pool,
            tc.tile_pool(name="ob", bufs=4) as out_pool,
            tc.tile_pool(name="psum", bufs=4, space="PSUM") as psum_pool,
        ):
            if mode == "bf16":
                vn_sb = vn_pool.tile([128, KT, LV], mdt)
                tn_sbs = [
                    tn_pool.tile([128, KT, wd], mdt, name=f"tn{p}_sb")
                    for p, wd in enumerate(WIDTHS)
                ]
            else:
                vn_sb = vn_pool.tile([128, KT2, 2, LV], mdt)
                tn_sbs = [
                    tn_pool.tile([128, KT2, 2, wd], mdt, name=f"tn{p}_sb")
                    for p, wd in enumerate(WIDTHS)
                ]

            def dma_tn(pi, kc, ch):
                if mode == "bf16":
                    nc.sync.dma_start(
                        tn_sbs[pi][:, kc : kc + ch, :],
                        tn_drams[pi][:, kc : kc + ch, :],
                    )
                else:
                    nc.sync.dma_start(
                        tn_sbs[pi][:, kc : kc + ch, :, :],
                        tn_drams[pi][:, kc : kc + ch, :, :],
                    )

            for pi, tms in enumerate(TM_PASSES):
                # full-bank psum tiles (psB only uses 64 cols): sharing a
                # bank across accumulation groups is unsafe because
                # start=True zeroes at zero-region granularity
                psA = {
                    tm: psum_pool.tile([128, 512], f32, name=f"psA_{tm}", tag="psA")
                    for tm in tms
                }
                psB = {
                    tm: psum_pool.tile([128, 512], f32, name=f"psB_{tm}", tag="psB")
                    for tm in tms
                }
                if pi == 0:
                    _warmup(nc, warm_pool, psA[tms[0]], mybir)
                kc = 0
                for ch in (CHUNKS if pi == 0 else (NK,)):
                    if pi == 0:
                        # vn chunk loads ride the scalar HWDGE queue
                        if mode == "bf16":
                            nc.scalar.dma_start(
                                vn_sb[:, kc : kc + ch, :], vnT[:, kc : kc + ch, :]
                            )
                        else:
                            nc.scalar.dma_start(
                                vn_sb[:, kc : kc + ch, :, :],
                                vnT[:, kc : kc + ch, :, :],
                            )
                        dma_tn(0, kc, ch)
                    for kk in range(ch):
                        k = kc + kk
                        for tmi, tm in enumerate(tms):
                            js = tmi * 128
                            # B (64-wide) first: the next group's LDWEIGHTS
                            # then hides under A's long 512-wide stream
                            if mode == "bf16":
                                nc.tensor.matmul(
                                    psB[tm][:, 0:64],
                                    lhsT=tn_sbs[pi][:, k, js : js + 128],
                                    rhs=vn_sb[:, k, 512:LV],
                                    start=(k == 0),
                                    stop=(k == NK - 1),
                                )
                                nc.tensor.matmul(
                                    psA[tm][:, 0:512],
                                    lhsT=tn_sbs[pi][:, k, js : js + 128],
                                    rhs=vn_sb[:, k, 0:512],
                                    start=(k == 0),
                                    stop=(k == NK - 1),
                                )
                            else:
                                nc.tensor.matmul(
                                    psB[tm][:, 0:64],
                                    lhsT=tn_sbs[pi][:, k, :, js : js + 128],
                                    rhs=vn_sb[:, k, :, 512:LV],
                                    start=(k == 0),
                                    stop=(k == NK - 1),
                                    perf_mode=mybir.MatmulPerfMode.DoubleRow,
                                )
                                nc.tensor.matmul(
                                    psA[tm][:, 0:512],
                                    lhsT=tn_sbs[pi][:, k, :, js : js + 128],
                                    rhs=vn_sb[:, k, :, 0:512],
                                    start=(k == 0),
                                    stop=(k == NK - 1),
                                    perf_mode=mybir.MatmulPerfMode.DoubleRow,
                                )
                    kc += ch
                if pi == 0:
                    # queue later passes' tn transfers on the sync ring NOW,
                    # ahead of this pass's output DMAs (whose triggers stall
                    # on the last matmul and would block the prefetch)
                    for np_ in range(1, len(TM_PASSES)):
                        for nch_c in range(0, NK, NK // 2):
                            dma_tn(np_, nch_c, NK // 2)
                # downcast finished psums to bf16 and stream out; pass-0
                # copies/DMAs overlap pass-1 matmuls. Casts split across the
                # vector and scalar engines so the final-pass tail is short.
                for i, tm in enumerate(tms):
                    ot = out_pool.tile([128, LV], bf16, name=f"ot_{tm}", tag="ot")
                    if i % 2 == 0:
                        nc.vector.tensor_copy(ot[:, 0:512], psA[tm][:, 0:512])
                        nc.scalar.copy(ot[:, 512:LV], psB[tm][:, 0:64])
                    else:
                        nc.scalar.copy(ot[:, 0:512], psA[tm][:, 0:512])
                        nc.vector.tensor_copy(ot[:, 512:LV], psB[tm][:, 0:64])
                    eng = nc.sync if i % 2 == 0 else nc.scalar
                    eng.dma_start(sc[tm], ot[:, :])

    nc.compile()
    return nc


def _build_pool_nc():
    """out[:, c*512:(c+1)*512] = (W @ vf) for this core's 512-column slice.

    Column sharding: each core gets the full (small) W but only a 512-wide
    slice of vf. All operands bf16; output bf16, upcast on host."""
    import concourse.mybir as mybir
    import concourse.tile as tile
    from concourse import bacc

    nc = bacc.Bacc(
        "TRN2",
        target_bir_lowering=False,
        debug=False,
        enable_asserts=True,
        num_devices=NCORES,
    )
    bf16 = mybir.dt.bfloat16
    f32 = mybir.dt.float32
    wT = nc.dram_tensor("wT", [KV, 128, LV], bf16, kind="ExternalInput").ap()
    vf = nc.dram_tensor("vf", [KV, 128, 512], bf16, kind="ExternalInput").ap()
    out = nc.dram_tensor("out", [LV, 512], bf16, kind="ExternalOutput").ap()

    with tile.TileContext(nc) as tc:
        with (
            tc.tile_pool(name="warm", bufs=1) as warm_pool,
            tc.tile_pool(name="w", bufs=1) as w_pool,
            tc.tile_pool(name="vfp", bufs=1) as vf_pool,
            tc.tile_pool(name="ob", bufs=5) as out_pool,
            tc.tile_pool(name="psum", bufs=5, space="PSUM") as psum_pool,
        ):
            w_sb = w_pool.tile([128, KV, LV], bf16)
            vf_sb = vf_pool.tile([128, KV, 512], bf16)
            psums = [
                psum_pool.tile([128, 512], f32, name=f"pps{m}", tag="pps")
                for m in range(len(M_TILES))
            ]
            _warmup(nc, warm_pool, psums[0], mybir, n_warm=7)
            # k-outer streaming: first k-tile lands, all 5 m-tiles consume it
            for k in range(KV):
                nc.scalar.dma_start(w_sb[:, k, :], wT[k])
                nc.sync.dma_start(vf_sb[:, k, :], vf[k])
                for m, pm in enumerate(M_TILES):
                    nc.tensor.matmul(
                        psums[m][:pm, :],
                        lhsT=w_sb[:, k, m * 128 : m * 128 + pm],
                        rhs=vf_sb[:, k, :],
                        start=(k == 0),
                        stop=(k == KV - 1),
                    )
            for m, pm in enumerate(M_TILES):
                ot = out_pool.tile([128, 512], bf16, name=f"pot{m}", tag="pot")
                if m % 2 == 0:
                    nc.vector.tensor_copy(ot[:pm, :], psums[m][:pm, :])
                else:
                    nc.scalar.copy(ot[:pm, :], psums[m][:pm, :])
                eng = nc.scalar if m % 2 == 0 else nc.sync
                eng.dma_start(out[m * 128 : m * 128 + pm, :], ot[:pm, :])

    nc.compile()
    return nc


def _get_nc(which: str):
    key = which
    if key not in _cache:
        if which == "scores":
            _cache[key] = _build_scores_nc(SCORES_MODE)
        else:
            _cache[key] = _build_pool_nc()
    return _cache[key]


class _Runner:
    """Cached PJRT executor for one Bass program across the 8 cores.

    Mirrors bass2jax.run_bass_via_pjrt's multi-core branch, but builds the
    jitted shard_map once (that function re-traces and re-compiles on every
    call) and lets chosen inputs be replicated instead of concatenated.

    Call with a dict: sharded inputs as global arrays (axis 0 = n_cores *
    per-core axis 0), replicated inputs at their per-core shape. Returns
    {name: global ndarray} with outputs concatenated along axis 0.
    """

    def __init__(self, nc, replicated=()):
        import jax
        from jax.experimental.shard_map import shard_map
        from jax.sharding import Mesh, PartitionSpec

        import concourse.mybir as mybir
        from concourse import bass2jax

        bass2jax.install_neuronx_cc_hook()
        assert not nc.has_collectives and nc.dbg_addr is None
        self.nc = nc
        part_name = nc.partition_id_tensor.name if nc.partition_id_tensor else None
        in_names, out_names, out_avals = [], [], []
        for alloc in nc.m.functions[0].allocations:
            if not isinstance(alloc, mybir.MemoryLocationSet):
                continue
            name = alloc.memorylocations[0].name
            if alloc.kind == "ExternalInput":
                if name != part_name:
                    in_names.append(name)
            elif alloc.kind == "ExternalOutput":
                out_names.append(name)
                out_avals.append(
                    jax.core.ShapedArray(
                        tuple(alloc.tensor_shape), mybir.dt.np(alloc.dtype)
                    )
                )
        self.in_names, self.out_names, self.out_avals = in_names, out_names, out_avals
        self.replicated = set(replicated)
        n_params = len(in_names)
        donate = tuple(range(n_params, n_params + len(out_names)))

        bind_names = in_names + out_names + ([part_name] if part_name else [])

        def _body(*args):
            operands = list(args)
            if part_name is not None:
                operands.append(bass2jax.partition_id_tensor())
            outs = bass2jax._bass_exec_p.bind(
                *operands,
                out_avals=tuple(out_avals),
                in_names=tuple(bind_names),
                out_names=tuple(out_names),
                lowering_input_output_aliases=(),
                sim_require_finite=True,
                sim_require_nnan=True,
                nc=nc,
            )
            return tuple(outs)

        devices = jax.devices()[:NCORES]
        mesh = Mesh(np.asarray(devices), ("core",))
        in_specs = tuple(
            PartitionSpec() if n in self.replicated else PartitionSpec("core")
            for n in in_names
        ) + (PartitionSpec("core"),) * len(out_names)
        out_specs = (PartitionSpec("core"),) * len(out_names)
        self._fn = jax.jit(
            shard_map(
                _body,
                mesh=mesh,
                in_specs=in_specs,
                out_specs=out_specs,
                check_rep=False,
            ),
            donate_argnums=donate,
            keep_unused=True,
        )

    def __call__(self, inputs: dict):
        args = [np.ascontiguousarray(inputs[n]) for n in self.in_names]
        zeros = [
            np.zeros((NCORES * a.shape[0], *a.shape[1:]), a.dtype)
            for a in self.out_avals
        ]
        outs = self._fn(*args, *zeros)
        return {n: np.asarray(o) for n, o in zip(self.out_names, outs)}


_runners: dict = {}


def _get_runner(which: str) -> _Runner:
    key = which
    if key not in _runners:
        repl = {"scores": ("vnT",), "pool": ("wT",)}[which]
        _runners[key] = _Runner(_get_nc(which), replicated=repl)
    return _runners[key]


def _neighbor_unique(sel: np.ndarray) -> np.ndarray:
    offs = np.array(
        [
            [i, j]
            for i in range(-PAD, PAD + 1)
            for j in range(-PAD, PAD + 1)
            if not (i == 0 and j == 0)
        ],
        dtype=np.int64,
    )
    coords = np.stack([sel // GRID, sel % GRID], axis=1)
    padded = np.clip(coords[:, None, :] + offs[None, :, :], 0, GRID - 1)
    return np.unique(padded[..., 0] * GRID + padded[..., 1])


def kernel(vision_feature, text_embed, attention_mask):
    import jax
    import jax.numpy as jnp

    cpu = jax.devices("cpu")[0]

    vision_feature = np.asarray(vision_feature, dtype=np.float32)
    text_embed = np.asarray(text_embed, dtype=np.float32)
    mask_np = np.asarray(attention_mask)

    with jax.default_device(cpu):
        # normalize exactly as the reference does (jnp on CPU)
        vfj = jnp.asarray(vision_feature)
        tej = jnp.asarray(text_embed)
        vn = np.asarray(
            vfj / jnp.maximum(jnp.linalg.norm(vfj, axis=-1, keepdims=True), EPS)
        )
        tn = np.asarray(
            tej / jnp.maximum(jnp.linalg.norm(tej, axis=-1, keepdims=True), EPS)
        )

    # fold the attention mask into the text rows: where(mask, cos, 0) ==
    # cos * mask elementwise, and max over the text dim commutes with the
    # per-vision positive scale, so pre-scaling text rows by mask is exact.
    tns = tn * mask_np.astype(np.float32)[:, None]

    # ---- device program 1: sharded cos-sim, full bf16 cos.T shards ----
    splits = []
    lo = 0
    for tms in TM_PASSES:
        splits.append((lo, lo + len(tms)))
        lo += len(tms)
    if SCORES_MODE == "bf16":
        mnp = ml_dtypes.bfloat16
        vnT = np.ascontiguousarray(
            vn.T.reshape(KT, 128, LV).transpose(1, 0, 2)
        ).astype(mnp)
        # tn5[c, p, k, tm, j] = tns[c*1024 + tm*128 + j, k*128 + p]
        tn5 = tns.reshape(NCORES, TM, 128, KT, 128).transpose(0, 4, 3, 1, 2)
        tn_in = {
            f"tn{p}": np.ascontiguousarray(tn5[:, :, :, a:b, :]).reshape(
                NCORES * 128, KT, (b - a) * 128
            ).astype(mnp)
            for p, (a, b) in enumerate(splits)
        }
    else:
        mnp = ml_dtypes.float8_e4m3
        vnT = np.ascontiguousarray(
            (vn.T * FP8_SCALE).reshape(KT2, 2, 128, LV).transpose(2, 0, 1, 3)
        ).astype(mnp)
        # tn6[c, p, t, i, tm, j] = tns[c*1024+tm*128+j, (t*2+i)*128 + p] * S
        tn6 = (tns * FP8_SCALE).reshape(NCORES, TM, 128, KT2, 2, 128).transpose(
            0, 5, 3, 4, 1, 2
        )
        tn_in = {
            f"tn{p}": np.ascontiguousarray(tn6[:, :, :, :, a:b, :]).reshape(
                NCORES * 128, KT2, 2, (b - a) * 128
            ).astype(mnp)
            for p, (a, b) in enumerate(splits)
        }

    out1 = _get_runner("scores")({"vnT": vnT, **tn_in})
    # sc[c, tm, t, v] -> cos[v, c*1024 + tm*128 + t]
    sc = out1["sc"].reshape(NCORES, TM, 128, LV)
    cos_noisy = sc.transpose(3, 0, 1, 2).reshape(LV, LT).astype(np.float32)

    # noisy top-NCAND text candidates per vision token, rescored exactly in
    # fp64: the true argmax is inside the noisy top-8 by a wide margin, so
    # the selection sees bit-exact scores regardless of matmul precision.
    cand = np.argpartition(-cos_noisy, NCAND - 1, axis=1)[:, :NCAND]  # [LV, NCAND]
    exact = np.einsum(
        "mkd,md->mk", tns[cand].astype(np.float64), vn.astype(np.float64)
    ).astype(np.float32)
    scores = exact.max(axis=1)  # [LV]

    # ---- host selection (mirrors reference ops; margins >> fp32 noise) ----
    with jax.default_device(cpu):
        sj = jnp.asarray(scores)
        probs = jax.nn.softmax(sj / TEMP)
        order = jnp.argsort(-probs)
        cum = jnp.cumsum(probs[order])
        thr = int(jnp.sum(cum <= GAMMA))
        sel = np.asarray(order[:thr])

    if thr == 0:
        return np.zeros((0, D), dtype=np.float32)
    uniq = _neighbor_unique(sel)
    S = len(uniq)

    # ---- host: small [S,576] cos-sim + top-k + softmax, bit-exact ----
    with jax.default_device(cpu):
        sel_feat = jnp.asarray(vision_feature[uniq])
        sn = sel_feat / jnp.maximum(
            jnp.linalg.norm(sel_feat, axis=-1, keepdims=True), EPS
        )
        scos = sn @ jnp.asarray(vn).T
        top_vals, top_idx = jax.lax.top_k(scos, TOP_K)
        w = np.asarray(jax.nn.softmax(top_vals, axis=-1))
        top_idx = np.asarray(top_idx)

    if not POOL_ON_DEVICE:
        # weighted pooling on host, exact fp32, same op order as the module
        return np.ascontiguousarray(
            (vision_feature[top_idx] * w[..., None]).sum(axis=1)
        )

    W = np.zeros((LV, LV), dtype=np.float32)  # rows: uniq order; cols: vision j
    W[np.arange(S)[:, None], top_idx] = w

    # ---- device program 2: out = W @ vision_feature, column-sharded ----
    WT = np.zeros((KV * 128, LV), dtype=np.float32)
    WT[:LV] = W.T
    wT_r = WT.reshape(KV, 128, LV).astype(ml_dtypes.bfloat16)  # replicated
    vf_p = np.zeros((KV * 128, D), dtype=np.float32)
    vf_p[:LV] = vision_feature
    # global vf[c*KV+k, p, j] = vf_p[k*128+p, c*512+j]
    vf_g = np.ascontiguousarray(
        vf_p.reshape(KV, 128, NCORES, 512).transpose(2, 0, 1, 3)
    ).reshape(NCORES * KV, 128, 512).astype(ml_dtypes.bfloat16)

    out2 = _get_runner("pool")({"wT": wT_r, "vf": vf_g})
    # out is [NCORES*576, 512]: per-core column slices of [576, 4096]
    out_full = (
        out2["out"].reshape(NCORES, LV, 512).transpose(1, 0, 2)
        .reshape(LV, D).astype(np.float32)
    )
    return np.ascontiguousarray(out_full[:S])


# revision 32
# speedup vs baseline: 1.5457x; 1.0132x over previous
# Trainium2 Bass kernel for nn_CosSimRouter_pad.
#
# Strategy (8 NeuronCores, SPMD, no collectives):
#   Device program 1 ("scores"): cos = normalize(vision) @ normalize(text).T
#     sharded over the text dim (1024 text rows per core). Text-stationary
#     matmul: stationary = 128-text-column tiles of the shard (always 128
#     wide, so the compiler's fast-weight-load path engages), moving = the
#     resident normalized vision matrix (576 wide = 512 + 64 psum split).
#     Runs in bf16 or fp8e4+DoubleRow; each core returns its [1024, 576]
#     cos.T shard in bf16. The host takes the noisy top-8 text candidates
#     per vision token and rescores them exactly in fp64, so matmul noise
#     never reaches the (discrete) selection stage.
#   Host: softmax/argsort/cumsum threshold selection, neighbor expansion,
#     unique, then the small [S,576] cos-sim + top-16 + softmax weights are
#     computed with jax on CPU using the exact op sequence of the original
#     module so the selection matches it bit-for-bit. The weights are
#     scattered into a dense row-sparse matrix W [576, 576].
#   Device program 2 ("pool"): out = W @ vision_feature in bf16, sharded
#     over output columns (512 per core), returned bf16, upcast on host.
#
# Both programs start with a short burst of dummy matmuls so the PE HAM
# clock-gate warms up (1.2 -> 2.4 GHz) while the first DMA chunks land.

import os

os.environ.setdefault("MYCRO_LOCAL_CACHE", "1")

import numpy as np
import ml_dtypes

GAMMA = 0.5
TEMP = 0.05
TOP_K = 16
PAD = 1
GRID = 24
EPS = 1e-8

LV = 576          # vision tokens
LT = 8192         # text tokens
D = 4096          # embed dim
NCORES = 8
LT_SH = LT // NCORES          # 1024 text rows per core
KT = D // 128                 # 32 contraction tiles
KT2 = KT // 2                 # 16 contraction pair-tiles (fp8 DoubleRow)
TM = LT_SH // 128             # 8 text tiles of 128 per core
# pass sizes descending: the last pass's cast+DMA tail covers one tile only
TM_PASSES = ((0, 1, 2, 3), (4, 5, 6), (7,))
M_TILES = (128, 128, 128, 128, 64)   # pool: 576 = 4*128 + 64
KV = 5                        # ceil(576/128) contraction tiles for program 2
NCAND = 8                     # noisy top-k candidates rescored exactly on host
FP8_SCALE = 64.0              # pre-scale so fp8e4 sees ~N(0,1) magnitudes
N_WARM = 8                    # dummy warm-up matmuls per program

# If False, the small final pooling (W @ vision_feature, 2.7 GFLOP = 6.5% of
# the module's FLOPs) is done on host in exact fp32 as part of the
# selection/gather stage; the device runs the dominant cos-sim matmul.
POOL_ON_DEVICE = False

# "bf16" or "fp8dr" (fp8e4 + DoubleRow, 2x matmul rate)
SCORES_MODE = "fp8dr"

_cache: dict = {}


def _warmup(nc, wsb_pool, warm_psum, mybir, n_warm=N_WARM):
    """Dummy matmuls at program start: PE busy while first DMAs land, so the
    HAM activity window un-throttles the clock before the real work. Writes
    into a real psum tile; the first real start=True matmul re-zeroes it."""
    wt = wsb_pool.tile([128, 512], mybir.dt.bfloat16)
    nc.vector.memset(wt[:, :], 0)
    for _ in range(n_warm):
        nc.tensor.matmul(
            warm_psum[:, :], lhsT=wt[:, :128], rhs=wt[:, :], start=True, stop=True
        )


def _build_scores_nc(mode: str):
    """Per text shard: full [1024, 576] cos.T matrix in bf16 (text-stationary)."""
    import concourse.mybir as mybir
    import concourse.tile as tile
    from concourse import bacc

    nc = bacc.Bacc(
        "TRN2",
        target_bir_lowering=False,
        debug=False,
        enable_asserts=True,
        num_devices=NCORES,
    )
    bf16 = mybir.dt.bfloat16
    f32 = mybir.dt.float32

    WIDTHS = tuple(len(tms) * 128 for tms in TM_PASSES)
    if mode == "bf16":
        mdt = bf16
        # partition-major: each SBUF partition's data is one contiguous DRAM
        # run, and each chunk is a single DMA trigger (the ~700ns per-trigger
        # issue cost on the sync engine was the early-stream bottleneck)
        vnT = nc.dram_tensor("vnT", [128, KT, LV], mdt, kind="ExternalInput").ap()
        tn_drams = [
            nc.dram_tensor(f"tn{p}", [128, KT, wd], mdt, kind="ExternalInput").ap()
            for p, wd in enumerate(WIDTHS)
        ]
        CHUNKS = (1, 1, 2, 4, 8, 8, 8)   # k-tiles per streamed chunk
        NK = KT
    else:  # fp8dr: k-tiles packed in pairs for DoubleRow
        mdt = mybir.dt.float8e4
        vnT = nc.dram_tensor("vnT", [128, KT2, 2, LV], mdt, kind="ExternalInput").ap()
        tn_drams = [
            nc.dram_tensor(
                f"tn{p}", [128, KT2, 2, wd], mdt, kind="ExternalInput"
            ).ap()
            for p, wd in enumerate(WIDTHS)
        ]
        CHUNKS = (1, 1, 1, 1, 2, 2, 2, 2, 4)   # pair-tiles per streamed chunk
        NK = KT2
    assert sum(CHUNKS) == NK

    # sc[tm, t, v] = cos[v, tm*128 + t]
    sc = nc.dram_tensor("sc", [TM, 128, LV], bf16, kind="ExternalOutput").ap()

    with tile.TileContext(nc) as tc:
        with (
            tc.tile_pool(name="warm", bufs=1) as warm_pool,
            tc.tile_pool(name="vn", bufs=1) as vn_pool,
            tc.tile_pool(name="tn", bufs=1) as tn_pool,
            tc.tile_pool(name="ob", bufs=4) as out_pool,
            tc.tile_pool(name="psum", bufs=4, space="PSUM") as psum_pool,
        ):
            if mode == "bf16":
                vn_sb = vn_pool.tile([128, KT, LV], mdt)
                tn_sbs = [
                    tn_pool.tile([128, KT, wd], mdt, name=f"tn{p}_sb")
                    for p, wd in enumerate(WIDTHS)
                ]
            else:
                vn_sb = vn_pool.tile([128, KT2, 2, LV], mdt)
                tn_sbs = [
                    tn_pool.tile([128, KT2, 2, wd], mdt, name=f"tn{p}_sb")
                    for p, wd in enumerate(WIDTHS)
                ]

            def dma_tn(pi, kc, ch):
                if mode == "bf16":
                    nc.sync.dma_start(
                        tn_sbs[pi][:, kc : kc + ch, :],
                        tn_drams[pi][:, kc : kc + ch, :],
                    )
                else:
                    nc.sync.dma_start(
                        tn_sbs[pi][:, kc : kc + ch, :, :],
                        tn_drams[pi][:, kc : kc + ch, :, :],
                    )

            for pi, tms in enumerate(TM_PASSES):
                # full-bank psum tiles (psB only uses 64 cols): sharing a
                # bank across accumulation groups is unsafe because
                # start=True zeroes at zero-region granularity
                psA = {
                    tm: psum_pool.tile([128, 512], f32, name=f"psA_{tm}", tag="psA")
                    for tm in tms
                }
                psB = {
                    tm: psum_pool.tile([128, 512], f32, name=f"psB_{tm}", tag="psB")
                    for tm in tms
                }
                if pi == 0:
                    _warmup(nc, warm_pool, psA[tms[0]], mybir)
                kc = 0
                for ch in (CHUNKS if pi == 0 else (NK,)):
                    if pi == 0:
                        # vn chunk loads ride the scalar HWDGE queue
                        if mode == "bf16":
                            nc.scalar.dma_start(
                                vn_sb[:, kc : kc + ch, :], vnT[:, kc : kc + ch, :]
                            )
                        else:
                            nc.scalar.dma_start(
                                vn_sb[:, kc : kc + ch, :, :],
                                vnT[:, kc : kc + ch, :, :],
                            )
                        dma_tn(0, kc, ch)
                    for kk in range(ch):
                        k = kc + kk
                        for tmi, tm in enumerate(tms):
                            js = tmi * 128
                            # B (64-wide) first: the next group's LDWEIGHTS
                            # then hides under A's long 512-wide stream
                            if mode == "bf16":
                                nc.tensor.matmul(
                                    psB[tm][:, 0:64],
                                    lhsT=tn_sbs[pi][:, k, js : js + 128],
                                    rhs=vn_sb[:, k, 512:LV],
                                    start=(k == 0),
                                    stop=(k == NK - 1),
                                )
                                nc.tensor.matmul(
                                    psA[tm][:, 0:512],
                                    lhsT=tn_sbs[pi][:, k, js : js + 128],
                                    rhs=vn_sb[:, k, 0:512],
                                    start=(k == 0),
                                    stop=(k == NK - 1),
                                )
                            else:
                                nc.tensor.matmul(
                                    psB[tm][:, 0:64],
                                    lhsT=tn_sbs[pi][:, k, :, js : js + 128],
                                    rhs=vn_sb[:, k, :, 512:LV],
                                    start=(k == 0),
                                    stop=(k == NK - 1),
                                    perf_mode=mybir.MatmulPerfMode.DoubleRow,
                                )
                                nc.tensor.matmul(
                                    psA[tm][:, 0:512],
                                    lhsT=tn_sbs[pi][:, k, :, js : js + 128],
                                    rhs=vn_sb[:, k, :, 0:512],
                                    start=(k == 0),
                                    stop=(k == NK - 1),
                                    perf_mode=mybir.MatmulPerfMode.DoubleRow,
                                )
                    kc += ch
                if pi == 0:
                    # queue later passes' tn transfers on the sync ring NOW,
                    # ahead of this pass's output DMAs (whose triggers stall
                    # on the last matmul and would block the prefetch)
                    for np_ in range(1, len(TM_PASSES)):
                        for nch_c in range(0, NK, NK // 2):
                            dma_tn(np_, nch_c, NK // 2)
                # downcast finished psums to bf16 and stream out; pass-0
                # copies/DMAs overlap pass-1 matmuls. Casts split across the
                # vector and scalar engines so the final-pass tail is short.
                for i, tm in enumerate(tms):
                    ot = out_pool.tile([128, LV], bf16, name=f"ot_{tm}", tag="ot")
                    if i % 2 == 0:
                        nc.vector.tensor_copy(ot[:, 0:512], psA[tm][:, 0:512])
                        nc.scalar.copy(ot[:, 512:LV], psB[tm][:, 0:64])
                    else:
                        nc.scalar.copy(ot[:, 0:512], psA[tm][:, 0:512])
                        nc.vector.tensor_copy(ot[:, 512:LV], psB[tm][:, 0:64])
                    eng = nc.sync if i % 2 == 0 else nc.scalar
                    eng.dma_start(sc[tm], ot[:, :])

    nc.compile()
    return nc


def _build_pool_nc():
    """out[:, c*512:(c+1)*512] = (W @ vf) for this core's 512-column slice.

    Column sharding: each core gets the full (small) W but only a 512-wide
    slice of vf. All operands bf16; output bf16, upcast on host."""
    import concourse.mybir as mybir
    import concourse.tile as tile
    from concourse import bacc

    nc = bacc.Bacc(
        "TRN2",
        target_bir_lowering=False,
        debug=False,
        enable_asserts=True,
        num_devices=NCORES,
    )
    bf16 = mybir.dt.bfloat16
    f32 = mybir.dt.float32
    wT = nc.dram_tensor("wT", [KV, 128, LV], bf16, kind="ExternalInput").ap()
    vf = nc.dram_tensor("vf", [KV, 128, 512], bf16, kind="ExternalInput").ap()
    out = nc.dram_tensor("out", [LV, 512], bf16, kind="ExternalOutput").ap()

    with tile.TileContext(nc) as tc:
        with (
            tc.tile_pool(name="warm", bufs=1) as warm_pool,
            tc.tile_pool(name="w", bufs=1) as w_pool,
            tc.tile_pool(name="vfp", bufs=1) as vf_pool,
            tc.tile_pool(name="ob", bufs=5) as out_pool,
            tc.tile_pool(name="psum", bufs=5, space="PSUM") as psum_pool,
        ):
            w_sb = w_pool.tile([128, KV, LV], bf16)
            vf_sb = vf_pool.tile([128, KV, 512], bf16)
            psums = [
                psum_pool.tile([128, 512], f32, name=f"pps{m}", tag="pps")
                for m in range(len(M_TILES))
            ]
            _warmup(nc, warm_pool, psums[0], mybir, n_warm=7)
            # k-outer streaming: first k-tile lands, all 5 m-tiles consume it
            for k in range(KV):
                nc.scalar.dma_start(w_sb[:, k, :], wT[k])
                nc.sync.dma_start(vf_sb[:, k, :], vf[k])
                for m, pm in enumerate(M_TILES):
                    nc.tensor.matmul(
                        psums[m][:pm, :],
                        lhsT=w_sb[:, k, m * 128 : m * 128 + pm],
                        rhs=vf_sb[:, k, :],
                        start=(k == 0),
                        stop=(k == KV - 1),
                    )
            for m, pm in enumerate(M_TILES):
                ot = out_pool.tile([128, 512], bf16, name=f"pot{m}", tag="pot")
                if m % 2 == 0:
                    nc.vector.tensor_copy(ot[:pm, :], psums[m][:pm, :])
                else:
                    nc.scalar.copy(ot[:pm, :], psums[m][:pm, :])
                eng = nc.scalar if m % 2 == 0 else nc.sync
                eng.dma_start(out[m * 128 : m * 128 + pm, :], ot[:pm, :])

    nc.compile()
    return nc


def _get_nc(which: str):
    key = which
    if key not in _cache:
        if which == "scores":
            _cache[key] = _build_scores_nc(SCORES_MODE)
        else:
            _cache[key] = _build_pool_nc()
    return _cache[key]


class _Runner:
    """Cached PJRT executor for one Bass program across the 8 cores.

    Mirrors bass2jax.run_bass_via_pjrt's multi-core branch, but builds the
    jitted shard_map once (that function re-traces and re-compiles on every
    call) and lets chosen inputs be replicated instead of concatenated.

    Call with a dict: sharded inputs as global arrays (axis 0 = n_cores *
    per-core axis 0), replicated inputs at their per-core shape. Returns
    {name: global ndarray} with outputs concatenated along axis 0.
    """

    def __init__(self, nc, replicated=()):
        import jax
        from jax.experimental.shard_map import shard_map
        from jax.sharding import Mesh, PartitionSpec

        import concourse.mybir as mybir
        from concourse import bass2jax

        bass2jax.install_neuronx_cc_hook()
        assert not nc.has_collectives and nc.dbg_addr is None
        self.nc = nc
        part_name = nc.partition_id_tensor.name if nc.partition_id_tensor else None
        in_names, out_names, out_avals = [], [], []
        for alloc in nc.m.functions[0].allocations:
            if not isinstance(alloc, mybir.MemoryLocationSet):
                continue
            name = alloc.memorylocations[0].name
            if alloc.kind == "ExternalInput":
                if name != part_name:
                    in_names.append(name)
            elif alloc.kind == "ExternalOutput":
                out_names.append(name)
                out_avals.append(
                    jax.core.ShapedArray(
                        tuple(alloc.tensor_shape), mybir.dt.np(alloc.dtype)
                    )
                )
        self.in_names, self.out_names, self.out_avals = in_names, out_names, out_avals
        self.replicated = set(replicated)
        n_params = len(in_names)
        donate = tuple(range(n_params, n_params + len(out_names)))

        bind_names = in_names + out_names + ([part_name] if part_name else [])

        def _body(*args):
            operands = list(args)
            if part_name is not None:
                operands.append(bass2jax.partition_id_tensor())
            outs = bass2jax._bass_exec_p.bind(
                *operands,
                out_avals=tuple(out_avals),
                in_names=tuple(bind_names),
                out_names=tuple(out_names),
                lowering_input_output_aliases=(),
                sim_require_finite=True,
                sim_require_nnan=True,
                nc=nc,
            )
            return tuple(outs)

        devices = jax.devices()[:NCORES]
        mesh = Mesh(np.asarray(devices), ("core",))
        in_specs = tuple(
            PartitionSpec() if n in self.replicated else PartitionSpec("core")
            for n in in_names
        ) + (PartitionSpec("core"),) * len(out_names)
        out_specs = (PartitionSpec("core"),) * len(out_names)
        self._fn = jax.jit(
            shard_map(
                _body,
                mesh=mesh,
                in_specs=in_specs,
                out_specs=out_specs,
                check_rep=False,
            ),
            donate_argnums=donate,
            keep_unused=True,
        )

    def __call__(self, inputs: dict):
        args = [np.ascontiguousarray(inputs[n]) for n in self.in_names]
        zeros = [
            np.zeros((NCORES * a.shape[0], *a.shape[1:]), a.dtype)
            for a in self.out_avals
        ]
        outs = self._fn(*args, *zeros)
        return {n: np.asarray(o) for n, o in zip(self.out_names, outs)}


_runners: dict = {}


def _get_runner(which: str) -> _Runner:
    key = which
    if key not in _runners:
        repl = {"scores": ("vnT",), "pool": ("wT",)}[which]
        _runners[key] = _Runner(_get_nc(which), replicated=repl)
    return _runners[key]


def _neighbor_unique(sel: np.ndarray) -> np.ndarray:
    offs = np.array(
        [
            [i, j]
            for i in range(-PAD, PAD + 1)
            for j in range(-PAD, PAD + 1)
            if not (i == 0 and j == 0)
        ],
        dtype=np.int64,
    )
    coords = np.stack([sel // GRID, sel % GRID], axis=1)
    padded = np.clip(coords[:, None, :] + offs[None, :, :], 0, GRID - 1)
    return np.unique(padded[..., 0] * GRID + padded[..., 1])


def kernel(vision_feature, text_embed, attention_mask):
    import jax
    import jax.numpy as jnp

    cpu = jax.devices("cpu")[0]

    vision_feature = np.asarray(vision_feature, dtype=np.float32)
    text_embed = np.asarray(text_embed, dtype=np.float32)
    mask_np = np.asarray(attention_mask)

    with jax.default_device(cpu):
        # normalize exactly as the reference does (jnp on CPU)
        vfj = jnp.asarray(vision_feature)
        tej = jnp.asarray(text_embed)
        vn = np.asarray(
            vfj / jnp.maximum(jnp.linalg.norm(vfj, axis=-1, keepdims=True), EPS)
        )
        tn = np.asarray(
            tej / jnp.maximum(jnp.linalg.norm(tej, axis=-1, keepdims=True), EPS)
        )

    # fold the attention mask into the text rows: where(mask, cos, 0) ==
    # cos * mask elementwise, and max over the text dim commutes with the
    # per-vision positive scale, so pre-scaling text rows by mask is exact.
    tns = tn * mask_np.astype(np.float32)[:, None]

    # ---- device program 1: sharded cos-sim, full bf16 cos.T shards ----
    splits = []
    lo = 0
    for tms in TM_PASSES:
        splits.append((lo, lo + len(tms)))
        lo += len(tms)
    if SCORES_MODE == "bf16":
        mnp = ml_dtypes.bfloat16
        vnT = np.ascontiguousarray(
            vn.T.reshape(KT, 128, LV).transpose(1, 0, 2)
        ).astype(mnp)
        # tn5[c, p, k, tm, j] = tns[c*1024 + tm*128 + j, k*128 + p]
        tn5 = tns.reshape(NCORES, TM, 128, KT, 128).transpose(0, 4, 3, 1, 2)
        tn_in = {
            f"tn{p}": np.ascontiguousarray(tn5[:, :, :, a:b, :]).reshape(
                NCORES * 128, KT, (b - a) * 128
            ).astype(mnp)
            for p, (a, b) in enumerate(splits)
        }
    else:
        mnp = ml_dtypes.float8_e4m3
        vnT = np.ascontiguousarray(
            (vn.T * FP8_SCALE).reshape(KT2, 2, 128, LV).transpose(2, 0, 1, 3)
        ).astype(mnp)
        # tn6[c, p, t, i, tm, j] = tns[c*1024+tm*128+j, (t*2+i)*128 + p] * S
        tn6 = (tns * FP8_SCALE).reshape(NCORES, TM, 128, KT2, 2, 128).transpose(
            0, 5, 3, 4, 1, 2
        )
        tn_in = {
            f"tn{p}": np.ascontiguousarray(tn6[:, :, :, :, a:b, :]).reshape(
                NCORES * 128, KT2, 2, (b - a) * 128
            ).astype(mnp)
            for p, (a, b) in enumerate(splits)
        }

    out1 = _get_runner("scores")({"vnT": vnT, **tn_in})
    # sc[c, tm, t, v] -> cos[v, c*1024 + tm*128 + t]
    sc = out1["sc"].reshape(NCORES, TM, 128, LV)
    cos_noisy = sc.transpose(3, 0, 1, 2).reshape(LV, LT).astype(np.float32)

    # noisy top-NCAND text candidates per vision token, rescored exactly in
    # fp64: the true argmax is inside the noisy top-8 by a wide margin, so
    # the selection sees bit-exact scores regardless of matmul precision.
    cand = np.argpartition(-cos_noisy, NCAND - 1, axis=1)[:, :NCAND]  # [LV, NCAND]
    exact = np.einsum(
        "mkd,md->mk", tns[cand].astype(np.float64), vn.astype(np.float64)
    ).astype(np.float32)
    scores = exact.max(axis=1)  # [LV]

    # ---- host selection (mirrors reference ops; margins >> fp32 noise) ----
    with jax.default_device(cpu):
        sj = jnp.asarray(scores)
        probs = jax.nn.softmax(sj / TEMP)
        order = jnp.argsort(-probs)
        cum = jnp.cumsum(probs[order])
        thr = int(jnp.sum(cum <= GAMMA))
        sel = np.asarray(order[:thr])

    if thr == 0:
        return np.zeros((0, D), dtype=np.float32)
    uniq = _neighbor_unique(sel)
    S = len(uniq)

    # ---- host: small [S,576] cos-sim + top-k + softmax, bit-exact ----
    with jax.default_device(cpu):
        sel_feat = jnp.asarray(vision_feature[uniq])
        sn = sel_feat / jnp.maximum(
            jnp.linalg.norm(sel_feat, axis=-1, keepdims=True), EPS
        )
        scos = sn @ jnp.asarray(vn).T
        top_vals, top_idx = jax.lax.top_k(scos, TOP_K)
        w = np.asarray(jax.nn.softmax(top_vals, axis=-1))
        top_idx = np.asarray(top_idx)

    if not POOL_ON_DEVICE:
        # weighted pooling on host, exact fp32, same op order as the module
        return np.ascontiguousarray(
            (vision_feature[top_idx] * w[..., None]).sum(axis=1)
        )

    W = np.zeros((LV, LV), dtype=np.float32)  # rows: uniq order; cols: vision j
    W[np.arange(S)[:, None], top_idx] = w

    # ---- device program 2: out = W @ vision_feature, column-sharded ----
    WT = np.zeros((KV * 128, LV), dtype=np.float32)
    WT[:LV] = W.T
    wT_r = WT.reshape(KV, 128, LV).astype(ml_dtypes.bfloat16)  # replicated
    vf_p = np.zeros((KV * 128, D), dtype=np.float32)
    vf_p[:LV] = vision_feature
    # global vf[c*KV+k, p, j] = vf_p[k*128+p, c*512+j]
    vf_g = np.ascontiguousarray(
        vf_p.reshape(KV, 128, NCORES, 512).transpose(2, 0, 1, 3)
    ).reshape(NCORES * KV, 128, 512).astype(ml_dtypes.bfloat16)

    out2 = _get_runner("pool")({"wT": wT_r, "vf": vf_g})
    # out is [NCORES*576, 512]: per-core column slices of [576, 4096]
    out_full = (
        out2["out"].reshape(NCORES, LV, 512).transpose(1, 0, 2)
        .reshape(LV, D).astype(np.float32)
    )
    return np.ascontiguousarray(out_full[:S])
